# revision 1
# baseline (speedup 1.0000x reference)
"""GCLConv (GNN message passing) Trainium2 kernel — 8-core edge-parallel.

Strategy:
  - Host: sort edges by destination (row); shard by destination node range
    across 8 cores (6272 nodes/core) => no cross-core reduction needed.
  - Device per core: transpose-mode bf16 dma_gather of h[row]/h[col]
    (feature-major [D, e] tiles, zero input transposes), edge MLP as PE
    matmuls with f32 PSUM accumulation, segment-sum via S-matrix matmul
    accumulated in PSUM per 128-node window, then the node MLP + residual.
  - sigmoid(x) = 0.5*tanh(x/2)+0.5 so Silu/Tanh/Copy share one ACT table set.
  - int16 gather indices: col table split in two halves (<32768 rows each);
    edges grouped by (col-half, window) with per-group padding made uniform
    across cores so one SPMD program serves all 8 cores.
"""
import sys

sys.path.insert(0, "/opt/trn_rl_repo")

import numpy as np
import ml_dtypes

import concourse.bass as bass
import concourse.bacc as bacc
import concourse.mybir as mybir
import concourse.tile as tile
from concourse import bass_utils

BF16 = ml_dtypes.bfloat16

N = 50000
E = 800000
D = 128
H = 128
P = 128
NCORES = 8
WIN = 128                  # nodes per aggregation window
NW = 49                    # windows per core
SHARD = WIN * NW           # 6272 nodes per core
NPAD = SHARD * NCORES      # 50176
COL_SPLIT = 25088          # col gather table split (both halves < 32768)
COL_HI = NPAD - COL_SPLIT  # 25088
GB = 32                    # tiles per gather batch (4096 indices)
NORM = 100.0

FP32 = mybir.dt.float32
BF = mybir.dt.bfloat16
I16 = mybir.dt.int16


def _idx_layout(idx_flat: np.ndarray) -> np.ndarray:
    """Pack int16 indices into the SWDGE layout [128, n/16]:
    index i -> partition i%16, col i//16, replicated across 8 groups."""
    n = idx_flat.shape[0]
    assert n % 16 == 0
    arr = idx_flat.reshape(n // 16, 16).T.astype(np.int16)  # [16, n/16]
    return np.tile(arr, (8, 1))                             # [128, n/16]


def _preprocess(h: np.ndarray, edge_index: np.ndarray):
    """Build per-core edge tiles + metadata. Returns host data dict."""
    row = np.asarray(edge_index[0], dtype=np.int64)
    col = np.asarray(edge_index[1], dtype=np.int64)

    core_of = row // SHARD
    half_of = (col >= COL_SPLIT).astype(np.int64)

    # group counts per (core, half, window)
    win_of = (row % SHARD) // WIN
    counts = np.zeros((NCORES, 2, NW), dtype=np.int64)
    np.add.at(counts, (core_of, half_of, win_of), 1)
    tiles_per_group = np.maximum(1, -(-counts // P))        # ceil, min 1
    T_hw = tiles_per_group.max(axis=0)                      # [2, NW] uniform
    NT = int(T_hw.sum())

    # per-core edge ordering: sort by (half, window)
    col_idx = np.empty((NCORES, 128, NT * 8), dtype=np.int16)
    row_idx = np.empty((NCORES, 128, NT * 8), dtype=np.int16)
    rel_row = np.empty((NCORES, 128, NT), dtype=BF16)
    for k in range(NCORES):
        m = core_of == k
        rk, ck, hk, wk = row[m] - k * SHARD, col[m], half_of[m], win_of[m]
        order = np.lexsort((wk, hk))
        rk, ck, hk, wk = rk[order], ck[order], hk[order], wk[order]
        # group boundaries
        cnt = np.zeros((2, NW), dtype=np.int64)
        np.add.at(cnt, (hk, wk), 1)
        rows_l, cols_l, rel_l = [], [], []
        pos = 0
        for hf in range(2):
            for w in range(NW):
                c = int(cnt[hf, w])
                npad_e = int(T_hw[hf, w]) * P - c
                r_g = rk[pos:pos + c]
                c_g = ck[pos:pos + c] - hf * COL_SPLIT
                rel_g = (r_g % WIN).astype(np.float32)
                pos += c
                if npad_e:
                    r_g = np.concatenate([r_g, np.zeros(npad_e, np.int64)])
                    c_g = np.concatenate([c_g, np.zeros(npad_e, np.int64)])
                    rel_g = np.concatenate(
                        [rel_g, np.full(npad_e, 255.0, np.float32)])
                rows_l.append(r_g)
                cols_l.append(c_g)
                rel_l.append(rel_g)
        r_all = np.concatenate(rows_l)
        c_all = np.concatenate(cols_l)
        rel_all = np.concatenate(rel_l)
        assert r_all.shape[0] == NT * P
        col_idx[k] = _idx_layout(c_all.astype(np.int16))
        row_idx[k] = _idx_layout(r_all.astype(np.int16))
        rel_row[k] = rel_all.reshape(NT, P).T.astype(BF16)

    # gather tables
    h_pad = np.zeros((NPAD, D), dtype=np.float32)
    h_pad[:N] = h
    h_bf = h_pad.astype(BF16)
    hA = np.ascontiguousarray(h_bf[:COL_SPLIT])
    hB = np.ascontiguousarray(h_bf[COL_SPLIT:])
    hrow = h_bf.reshape(NCORES, SHARD, D)                   # per-core shard

    # node-phase buffers per core
    hsh = h_pad.reshape(NCORES, NW, WIN, D)
    h_own = np.ascontiguousarray(
        hsh.transpose(0, 2, 1, 3).reshape(NCORES, WIN, NW * D))  # [128, w*128+d]
    hT = np.ascontiguousarray(
        hsh.transpose(0, 3, 1, 2).reshape(NCORES, D, NW * WIN)).astype(BF16)

    return dict(NT=NT, T_hw=T_hw, col_idx=col_idx, row_idx=row_idx,
                rel_row=rel_row, hA=hA, hB=hB, hrow=hrow,
                h_own=h_own.astype(np.float32), hT=hT)


def _build(nc: bass.Bass, NT: int, T_hw: np.ndarray,
           act_silu, act_tanh):
    """Emit the SPMD program. T_hw: [2, NW] tiles per (half, window)."""
    dt = nc.dram_tensor
    hA_t = dt("hA", [COL_SPLIT, D], BF, kind="ExternalInput")
    hB_t = dt("hB", [COL_HI, D], BF, kind="ExternalInput")
    hrow_t = dt("hrow", [SHARD, D], BF, kind="ExternalInput")
    cidx_t = dt("col_idx", [128, NT * 8], I16, kind="ExternalInput")
    ridx_t = dt("row_idx", [128, NT * 8], I16, kind="ExternalInput")
    rel_t = dt("rel_row", [128, NT], BF, kind="ExternalInput")
    hown_t = dt("h_own", [WIN, NW * D], FP32, kind="ExternalInput")
    hT_t = dt("hT", [D, NW * WIN], BF, kind="ExternalInput")
    # weights / consts (replicated)
    eW1t_t = dt("eW1top", [D, H], BF, kind="ExternalInput")
    eW1b_t = dt("eW1bot", [D, H], BF, kind="ExternalInput")
    eW2_t = dt("eW2", [H, H], BF, kind="ExternalInput")
    combo_t = dt("combo", [H, H + 1], BF, kind="ExternalInput")  # [I | aW]
    nW1t_t = dt("nW1top", [D, H], BF, kind="ExternalInput")
    nW1b_t = dt("nW1bot", [H, H], BF, kind="ExternalInput")      # / NORM
    nW2_t = dt("nW2", [H, D], BF, kind="ExternalInput")
    ones_t = dt("ones_row", [1, WIN], BF, kind="ExternalInput")
    nb2_t = dt("nb2_row", [1, D], BF, kind="ExternalInput")
    eb1_t = dt("eb1", [H, 1], FP32, kind="ExternalInput")
    eb2_t = dt("eb2", [H, 1], FP32, kind="ExternalInput")
    nb1_t = dt("nb1", [H, 1], FP32, kind="ExternalInput")
    jconst_t = dt("jconst", [P, WIN], BF, kind="ExternalInput")
    ident_t = dt("ident", [P, P], FP32, kind="ExternalInput")
    ab_t = dt("ab_c", [P, 1], FP32, kind="ExternalInput")  # 0.5*ab per-partition

    out_t = dt("out", [SHARD, D], FP32, kind="ExternalOutput")

    with tile.TileContext(nc) as tc:
        with (
            tc.tile_pool(name="const", bufs=1) as cp,
            tc.tile_pool(name="gather", bufs=3) as gp,
            tc.tile_pool(name="work", bufs=3) as wp,
            tc.tile_pool(name="agg", bufs=1) as ap_,
            tc.tile_pool(name="ps", bufs=2, space="PSUM") as ps,
            tc.tile_pool(name="psagg", bufs=2, space="PSUM") as psg,
        ):
            # --- resident uploads ---
            def up(shape, dtype, src, tag):
                t = cp.tile(shape, dtype, tag=tag)
                nc.sync.dma_start(out=t[:], in_=src[:])
                return t

            cidx = up([128, NT * 8], I16, cidx_t, "cidx")
            ridx = up([128, NT * 8], I16, ridx_t, "ridx")
            rel = up([128, NT], BF, rel_t, "rel")
            h_own = up([WIN, NW * D], FP32, hown_t, "hown")
            hT = up([D, NW * WIN], BF, hT_t, "hT")
            eW1t = up([D, H], BF, eW1t_t, "eW1t")
            eW1b = up([D, H], BF, eW1b_t, "eW1b")
            eW2 = up([H, H], BF, eW2_t, "eW2")
            combo = up([H, H + 1], BF, combo_t, "combo")
            nW1t = up([D, H], BF, nW1t_t, "nW1t")
            nW1b = up([H, H], BF, nW1b_t, "nW1b")
            nW2 = up([H, D], BF, nW2_t, "nW2")
            ones_r = up([1, WIN], BF, ones_t, "ones")
            nb2_r = up([1, D], BF, nb2_t, "nb2")
            eb1 = up([H, 1], FP32, eb1_t, "eb1")
            eb2 = up([H, 1], FP32, eb2_t, "eb2")
            nb1 = up([H, 1], FP32, nb1_t, "nb1")
            jconst = up([P, WIN], BF, jconst_t, "jconst")
            ident = up([P, P], FP32, ident_t, "ident")
            ab_c = up([P, 1], FP32, ab_t, "ab")

            agg = ap_.tile([WIN, NW * H], FP32)  # node-major agg per window

            # --- tile schedule: (half, window) groups; batched gathers ---
            sched = []  # (tile_idx, half, window, first_in_group, last_in_group)
            t_i = 0
            for hf in range(2):
                for w in range(NW):
                    n_t = int(T_hw[hf, w])
                    for i in range(n_t):
                        sched.append((t_i, hf, w, i == 0, i == n_t - 1))
                        t_i += 1
            assert t_i == NT
            TA = int(T_hw[0].sum())  # tiles in half 0

            # batches never cross the half boundary
            batches = []
            for lo, hi in ((0, TA), (TA, NT)):
                b = lo
                while b < hi:
                    batches.append((b, min(GB, hi - b)))
                    b += GB

            bt_of_tile = {}
            for bi, (b0, nb) in enumerate(batches):
                for j in range(nb):
                    bt_of_tile[b0 + j] = (bi, j)

            gtiles = {}

            def emit_batch(bi):
                b0, nb = batches[bi]
                half = 0 if b0 < TA else 1
                src = hA_t if half == 0 else hB_t
                ct = gp.tile([128, GB * P], BF, tag="gcol")
                rt = gp.tile([128, GB * P], BF, tag="grow")
                for (buf, src_ap, idx) in ((ct, src, cidx), (rt, hrow_t, ridx)):
                    nc.gpsimd.dma_gather(
                        out_ap=buf[:, :nb * P].rearrange("p (a n) -> p a n", a=1),
                        in_ap=src_ap[:],
                        idxs_ap=idx[:, b0 * 8:(b0 + nb) * 8],
                        num_idxs=nb * P,
                        num_idxs_reg=nb * P,
                        elem_size=D,
                        transpose=True,
                        single_packet=False,
                    )
                gtiles[bi] = (ct, rt)

            emit_batch(0)
            for (t, hf, w, first, last) in sched:
                bi, j = bt_of_tile[t]
                if j == 0 and bi + 1 < len(batches):
                    emit_batch(bi + 1)
                ct, rt = gtiles[bi]
                es = slice(j * P, (j + 1) * P)

                ps1 = ps.tile([H, P], FP32, space="PSUM", tag="ps1")
                nc.tensor.matmul(ps1[:], lhsT=eW1t[:], rhs=rt[:, es],
                                 start=True, stop=False)
                nc.tensor.matmul(ps1[:], lhsT=eW1b[:], rhs=ct[:, es],
                                 start=False, stop=True)
                m1 = wp.tile([H, P], BF, tag="m1")
                nc.scalar.activation(m1[:], ps1[:], act_silu, bias=eb1[:])

                ps2 = ps.tile([H, P], FP32, space="PSUM", tag="ps2")
                nc.tensor.matmul(ps2[:], lhsT=eW2[:], rhs=m1[:],
                                 start=True, stop=True)
                m2 = wp.tile([H, P], BF, tag="m2")
                nc.scalar.activation(m2[:], ps2[:], act_silu, bias=eb2[:])

                # [m2_edge_major | att_pre] = m2T.T @ [I | aW]
                ps3 = ps.tile([P, H + 1], FP32, space="PSUM", tag="ps3")
                nc.tensor.matmul(ps3[:], lhsT=m2[:], rhs=combo[:],
                                 start=True, stop=True)
                att_t = wp.tile([P, 1], FP32, tag="att_t")
                nc.scalar.activation(att_t[:], ps3[:, H:H + 1], act_tanh,
                                     bias=ab_c[:], scale=0.5)
                att = wp.tile([P, 1], FP32, tag="att")
                nc.scalar.activation(att[:], att_t[:],
                                     mybir.ActivationFunctionType.Copy,
                                     bias=0.5, scale=0.5)
                ef = wp.tile([P, H], BF, tag="ef")
                nc.vector.tensor_scalar_mul(ef[:], ps3[:, :H], att[:])

                S = wp.tile([P, WIN], BF, tag="S")
                nc.vector.tensor_tensor(
                    out=S[:], in0=rel[:, t:t + 1].to_broadcast([P, WIN]),
                    in1=jconst[:], op=mybir.AluOpType.is_equal)

                if first:
                    pagg_cur = psg.tile([WIN, H], FP32, space="PSUM", tag="pagg")
                pagg = pagg_cur
                nc.tensor.matmul(pagg[:], lhsT=S[:], rhs=ef[:],
                                 start=first, stop=last)
                if last:
                    wslice = slice(w * H, (w + 1) * H)
                    if hf == 0:
                        nc.vector.tensor_copy(agg[:, wslice], pagg[:])
                    else:
                        nc.vector.tensor_add(
                            out=agg[:, wslice], in0=agg[:, wslice], in1=pagg[:])

            # --- node phase ---
            for w in range(NW):
                wsl = slice(w * H, (w + 1) * H)
                aggT_ps = ps.tile([H, WIN], FP32, space="PSUM", tag="ps1")
                nc.tensor.transpose(aggT_ps[:], in_=agg[:, wsl], identity=ident[:])
                aggT = wp.tile([H, WIN], BF, tag="m1")
                nc.vector.tensor_copy(aggT[:], aggT_ps[:])

                psn1 = ps.tile([H, WIN], FP32, space="PSUM", tag="ps2")
                nc.tensor.matmul(psn1[:], lhsT=nW1t[:], rhs=hT[:, wsl],
                                 start=True, stop=False)
                nc.tensor.matmul(psn1[:], lhsT=nW1b[:], rhs=aggT[:],
                                 start=False, stop=True)
                y1 = wp.tile([H, WIN], BF, tag="m2")
                nc.scalar.activation(y1[:], psn1[:], act_silu, bias=nb1[:])

                psn2 = ps.tile([WIN, D], FP32, space="PSUM", tag="ps3")
                nc.tensor.matmul(psn2[:], lhsT=y1[:], rhs=nW2[:],
                                 start=True, stop=False)
                nc.tensor.matmul(psn2[:], lhsT=ones_r[:], rhs=nb2_r[:],
                                 start=False, stop=True)
                o_sb = wp.tile([WIN, D], FP32, tag="osb")
                nc.vector.tensor_add(out=o_sb[:], in0=psn2[:], in1=h_own[:, wsl])
                nc.sync.dma_start(out=out_t[w * WIN:(w + 1) * WIN, :], in_=o_sb[:])
    return nc


def _make_in_maps(prep, inputs):
    eW1 = np.asarray(inputs["eW1"], np.float32)
    aW = np.asarray(inputs["aW"], np.float32)
    nW1 = np.asarray(inputs["nW1"], np.float32)
    combo = np.concatenate([np.eye(H, dtype=np.float32),
                            aW.reshape(H, 1)], axis=1)
    jconst = np.broadcast_to(np.arange(WIN, dtype=np.float32)[None, :], (P, WIN))
    common = {
        "hA": prep["hA"], "hB": prep["hB"],
        "eW1top": eW1[:D].astype(BF16), "eW1bot": eW1[D:].astype(BF16),
        "eW2": np.asarray(inputs["eW2"], np.float32).astype(BF16),
        "combo": combo.astype(BF16),
        "nW1top": nW1[:D].astype(BF16),
        "nW1bot": (nW1[D:] / NORM).astype(BF16),
        "nW2": np.asarray(inputs["nW2"], np.float32).astype(BF16),
        "ones_row": np.ones((1, WIN), BF16),
        "nb2_row": np.asarray(inputs["nb2"], np.float32).reshape(1, D).astype(BF16),
        "eb1": np.asarray(inputs["eb1"], np.float32).reshape(H, 1),
        "eb2": np.asarray(inputs["eb2"], np.float32).reshape(H, 1),
        "nb1": np.asarray(inputs["nb1"], np.float32).reshape(H, 1),
        "jconst": np.ascontiguousarray(jconst).astype(BF16),
        "ident": np.eye(P, dtype=np.float32),
        # tanh form: sigmoid(x+ab) = 0.5*tanh(0.5x + 0.5ab) + 0.5
        "ab_c": np.full((P, 1), 0.5 * float(np.asarray(inputs["ab"]).ravel()[0]),
                        dtype=np.float32),
    }
    in_maps = []
    for k in range(NCORES):
        m = dict(common)
        m["hrow"] = np.ascontiguousarray(prep["hrow"][k])
        m["col_idx"] = np.ascontiguousarray(prep["col_idx"][k])
        m["row_idx"] = np.ascontiguousarray(prep["row_idx"][k])
        m["rel_row"] = np.ascontiguousarray(prep["rel_row"][k])
        m["h_own"] = np.ascontiguousarray(prep["h_own"][k])
        m["hT"] = np.ascontiguousarray(prep["hT"][k])
        in_maps.append(m)
    return in_maps


_RUN_KW = {}


def kernel(**inputs) -> np.ndarray:
    h = np.asarray(inputs["h"], np.float32)
    prep = _preprocess(h, np.asarray(inputs["edge_index"]))

    nc = bacc.Bacc("TRN2", target_bir_lowering=False, debug=False,
                   num_devices=NCORES)
    _build(nc, prep["NT"], prep["T_hw"],
           act_silu=mybir.ActivationFunctionType.Silu,
           act_tanh=mybir.ActivationFunctionType.Tanh)
    nc.compile()

    in_maps = _make_in_maps(prep, inputs)
    res = bass_utils.run_bass_kernel_spmd(
        nc, in_maps, core_ids=list(range(NCORES)), **_RUN_KW)
    out = np.empty((NPAD, D), dtype=np.float32)
    for k in range(NCORES):
        out[k * SHARD:(k + 1) * SHARD] = np.asarray(res.results[k]["out"])
    kernel._last_results = res
    return out[:N]



# revision 2
# speedup vs baseline: 1.9304x; 1.9304x over previous
"""GCLConv (GNN message passing) Trainium2 kernel v2 — 8-core edge-parallel.

v2 strategy (vs v1):
  - Row-side h[row] gather ELIMINATED: rows within a 128-node window are
    expanded via PE matmul (lhsT = A_w = h_w @ eW1t, rhs = S^T one-hot),
    halving SWDGE gather descriptors.
  - 512-edge supertiles (4 chunks of 128): big matmuls/activations at
    free-dim 512 amortize per-instruction overhead.
  - att folded into the aggregation S matrix (S_att = S_eq * att), so the
    edge features need no separate ef multiply; aggregation consumes the
    PE-transposed m2 directly.
  - Node phase fused per window off the PSUM aggregation (no agg buffer).
  - Gather modes: 'hbm2' dual col tables (cols split <32768 for int16),
    'hbm1' single table with sign-extended negative indices (base shifted
    by 32768 rows), 'sbuf2' SBUF-resident dual tables.
"""
import sys

sys.path.insert(0, "/opt/trn_rl_repo")

import numpy as np
import ml_dtypes

import concourse.bass as bass
import concourse.bacc as bacc
import concourse.mybir as mybir
import concourse.tile as tile
from concourse import bass_utils

BF16 = ml_dtypes.bfloat16

N = 50000
E = 800000
D = 128
H = 128
P = 128
NCORES = 8
WIN = 128
NW = 49
SHARD = WIN * NW           # 6272
NPAD = SHARD * NCORES      # 50176
COL_SPLIT = 25088
COL_HI = NPAD - COL_SPLIT  # 25088
NORM = 100.0
GBC = 32                   # chunks per gather batch (4096 indices)

MODE = "hbm2"              # 'hbm2' | 'hbm1' | 'sbuf2'
SAFE2D = False             # per-chunk 2-dim DVE builds instead of 3-dim bcast

FP32 = mybir.dt.float32
BF = mybir.dt.bfloat16
I16 = mybir.dt.int16


def _idx_layout(idx_flat: np.ndarray) -> np.ndarray:
    n = idx_flat.shape[0]
    assert n % 16 == 0
    arr = idx_flat.reshape(n // 16, 16).T.astype(np.int16)
    return np.tile(arr, (8, 1))


def _preprocess(h: np.ndarray, edge_index: np.ndarray, mode: str):
    """Group edges per core by (window[, col-half]); build tables."""
    row = np.asarray(edge_index[0], dtype=np.int64)
    col = np.asarray(edge_index[1], dtype=np.int64)
    two_regions = mode in ("hbm2", "sbuf2")

    core_of = row // SHARD
    win_of = (row % SHARD) // WIN
    half_of = (col >= COL_SPLIT).astype(np.int64) if two_regions else \
        np.zeros_like(col)
    nhf = 2 if two_regions else 1

    counts = np.zeros((NCORES, NW, nhf), dtype=np.int64)
    np.add.at(counts, (core_of, win_of, half_of), 1)
    nt = -(-counts // P)                       # ceil
    nt = nt.max(axis=0)                        # [NW, nhf] uniform over cores
    # every window needs >= 1 chunk for its aggregation group
    for w in range(NW):
        if nt[w].sum() == 0:
            nt[w, 0] = 1
    # pad total chunks to a multiple of 4 (supertiles) in the last group
    NT = int(nt.sum())
    extra = (-NT) % 4
    nt[NW - 1, nhf - 1] += extra
    NT += extra

    # processing order: (w, hf, j); region order: (hf, w, j)
    chunks = []                                # (w, hf)
    for w in range(NW):
        for hf in range(nhf):
            chunks += [(w, hf)] * int(nt[w, hf])
    assert len(chunks) == NT
    # region-local index of each chunk, and region chunk lists
    reg_len = [0] * nhf
    reg_idx = []
    for (w, hf) in chunks:
        reg_idx.append(reg_len[hf])
        reg_len[hf] += 1

    # per-core tables
    cidx = np.zeros((NCORES, 128, NT * 8), dtype=np.int16)
    rel_sb = np.zeros((NCORES, 128, NT), dtype=BF16)
    NST = NT // 4
    r2cols = NST * 512
    rel2 = np.zeros((NCORES, 1, r2cols), dtype=BF16)

    for k in range(NCORES):
        m = core_of == k
        rk = (row[m] - k * SHARD)
        ck = col[m]
        wk, hk = win_of[m], half_of[m]
        order = np.lexsort((hk, wk))
        rk, ck, wk, hk = rk[order], ck[order], wk[order], hk[order]
        cnt = np.zeros((NW, nhf), dtype=np.int64)
        np.add.at(cnt, (wk, hk), 1)
        # build padded per-chunk col/rel arrays in processing order,
        # and region-ordered col arrays for the gather table
        rel_all = np.full((NT, P), 255.0, dtype=np.float32)
        col_all = np.zeros((NT, P), dtype=np.int64)
        pos = 0
        ci = 0
        for w in range(NW):
            for hf in range(nhf):
                c = int(cnt[w, hf])
                ntg = int(nt[w, hf])
                r_g = rk[pos:pos + c] % WIN
                c_g = ck[pos:pos + c]
                pos += c
                flat_r = np.full(ntg * P, 255.0, np.float32)
                # pads must stay in-range AFTER the per-half offset
                flat_c = np.full(ntg * P, hf * COL_SPLIT, np.int64)
                flat_r[:c] = r_g
                flat_c[:c] = c_g
                rel_all[ci:ci + ntg] = flat_r.reshape(ntg, P)
                col_all[ci:ci + ntg] = flat_c.reshape(ntg, P)
                ci += ntg
        assert pos == rk.shape[0] and ci == NT
        # rel tables (processing order)
        rel_sb[k] = rel_all.T.astype(BF16)             # [128, NT]
        rel2[k, 0] = rel_all.reshape(-1).astype(BF16)
        # gather index table in REGION order
        gcols = np.zeros((NT, P), dtype=np.int64)
        # region base offsets in the table: region0 first
        base = [0, reg_len[0]] if nhf == 2 else [0]
        for t, (w, hf) in enumerate(chunks):
            gi = base[hf] + reg_idx[t]
            cc = col_all[t]
            if mode == "hbm1":
                gcols[gi] = cc - 32768
            else:
                gcols[gi] = cc - hf * COL_SPLIT
        cidx[k] = _idx_layout(gcols.reshape(-1).astype(np.int16))

    # gather source tables
    h_pad = np.zeros((NPAD, D), dtype=np.float32)
    h_pad[:N] = h
    h_bf = h_pad.astype(BF16)
    tabs = {}
    if mode == "hbm1":
        tabs["hfull"] = h_bf
    else:
        tabs["hA"] = np.ascontiguousarray(h_bf[:COL_SPLIT])
        tabs["hB"] = np.ascontiguousarray(h_bf[COL_SPLIT:])
    if mode == "sbuf2":
        def sb_layout(t):
            nr = t.shape[0] // 128
            return np.ascontiguousarray(
                t.reshape(nr, 128, D).transpose(1, 0, 2).reshape(128, nr * D))
        tabs["hA_s"] = sb_layout(tabs.pop("hA"))
        tabs["hB_s"] = sb_layout(tabs.pop("hB"))

    # node-phase per-core tables
    hsh = h_pad.reshape(NCORES, NW, WIN, D)
    h_own = np.ascontiguousarray(
        hsh.transpose(0, 2, 1, 3).reshape(NCORES, WIN, NW * D))
    hT = np.ascontiguousarray(
        hsh.transpose(0, 3, 1, 2).reshape(NCORES, D, NW * WIN)).astype(BF16)

    return dict(NT=NT, nt=nt, chunks=chunks, reg_idx=reg_idx,
                reg_len=reg_len, nhf=nhf, cidx=cidx, rel_sb=rel_sb,
                rel2=rel2, tabs=tabs, h_own=h_own, hT=hT, NST=NST,
                r2cols=r2cols)


def _build(nc: bass.Bass, prep, mode: str):
    NT, nt, chunks = prep["NT"], prep["nt"], prep["chunks"]
    reg_idx, reg_len, nhf = prep["reg_idx"], prep["reg_len"], prep["nhf"]
    NST, r2cols = prep["NST"], prep["r2cols"]
    act_silu = mybir.ActivationFunctionType.Silu
    act_tanh = mybir.ActivationFunctionType.Tanh
    act_copy = mybir.ActivationFunctionType.Copy

    dt = nc.dram_tensor
    if mode == "hbm1":
        hfull_t = dt("hfull", [NPAD, D], BF, kind="ExternalInput")
        src_ap = [hfull_t[32768:, :]]
    elif mode == "hbm2":
        hA_t = dt("hA", [COL_SPLIT, D], BF, kind="ExternalInput")
        hB_t = dt("hB", [COL_HI, D], BF, kind="ExternalInput")
        src_ap = [hA_t[:], hB_t[:]]
    else:  # sbuf2
        hAs_t = dt("hA_s", [128, (COL_SPLIT // 128) * D], BF,
                   kind="ExternalInput")
        hBs_t = dt("hB_s", [128, (COL_HI // 128) * D], BF,
                   kind="ExternalInput")
    cidx_t = dt("cidx", [128, NT * 8], I16, kind="ExternalInput")
    rel_t = dt("rel_sb", [128, NT], BF, kind="ExternalInput")
    rel2_t = dt("rel2", [1, r2cols], BF, kind="ExternalInput")
    hT_t = dt("hT", [D, NW * WIN], BF, kind="ExternalInput")
    hown_t = dt("h_own", [WIN, NW * D], FP32, kind="ExternalInput")
    # weights / consts
    eW1t_t = dt("eW1t", [D, H], BF, kind="ExternalInput")
    eW1b_t = dt("eW1b", [D, H], BF, kind="ExternalInput")
    eW2_t = dt("eW2", [H, H], BF, kind="ExternalInput")
    aW_t = dt("aW_col", [H, 1], BF, kind="ExternalInput")
    nW1t_t = dt("nW1t", [D, H], BF, kind="ExternalInput")
    nW1b_t = dt("nW1b_n", [H, H], BF, kind="ExternalInput")
    nW2_t = dt("nW2", [H, D], BF, kind="ExternalInput")
    ones_r_t = dt("ones_row", [1, WIN], BF, kind="ExternalInput")
    ones_c_t = dt("ones_col", [1, P], BF, kind="ExternalInput")
    nb2_t = dt("nb2_row", [1, D], BF, kind="ExternalInput")
    identb_t = dt("ident_bf", [P, P], BF, kind="ExternalInput")
    jconst4_t = dt("jconst4", [P, 4 * WIN], BF, kind="ExternalInput")
    jconstT_t = dt("jconstT", [P, 1], FP32, kind="ExternalInput")
    eb1_t = dt("eb1", [H, 1], FP32, kind="ExternalInput")
    eb2_t = dt("eb2", [H, 1], FP32, kind="ExternalInput")
    nb1_t = dt("nb1", [H, 1], FP32, kind="ExternalInput")
    ab_t = dt("ab_c", [P, 1], FP32, kind="ExternalInput")
    out_t = dt("out", [SHARD, D], FP32, kind="ExternalOutput")

    with tile.TileContext(nc) as tc:
        with (
            tc.tile_pool(name="const", bufs=1) as cp,
            tc.tile_pool(name="gat", bufs=3 if nhf == 1 else 2) as gp,
            tc.tile_pool(name="work", bufs=3) as wp,
            tc.tile_pool(name="how", bufs=2) as hop,
            tc.tile_pool(name="r2s", bufs=2) as rp,
            tc.tile_pool(name="pp", bufs=1, space="PSUM") as pp,
            tc.tile_pool(name="pagg", bufs=2, space="PSUM") as pgp,
        ):
            def up(shape, dtype, src, tag):
                t = cp.tile(shape, dtype, tag=tag)
                nc.sync.dma_start(out=t[:], in_=src[:])
                return t

            cidx = up([128, NT * 8], I16, cidx_t, "cidx")
            rel_sb = up([128, NT], BF, rel_t, "rel")
            hT = up([D, NW * WIN], BF, hT_t, "hT")
            eW1t = up([D, H], BF, eW1t_t, "eW1t")
            eW1b = up([D, H], BF, eW1b_t, "eW1b")
            eW2 = up([H, H], BF, eW2_t, "eW2")
            aW = up([H, 1], BF, aW_t, "aW")
            nW1t = up([D, H], BF, nW1t_t, "nW1t")
            nW1b = up([H, H], BF, nW1b_t, "nW1b")
            nW2 = up([H, D], BF, nW2_t, "nW2")
            ones_r = up([1, WIN], BF, ones_r_t, "onesr")
            ones_c = up([1, P], BF, ones_c_t, "onesc")
            nb2_r = up([1, D], BF, nb2_t, "nb2")
            identb = up([P, P], BF, identb_t, "identb")
            jconst4 = up([P, 4 * WIN], BF, jconst4_t, "jc4")
            jconstT = up([P, 1], FP32, jconstT_t, "jcT")
            eb1 = up([H, 1], FP32, eb1_t, "eb1")
            eb2 = up([H, 1], FP32, eb2_t, "eb2")
            nb1 = up([H, 1], FP32, nb1_t, "nb1")
            ab_c = up([P, 1], FP32, ab_t, "ab")
            if mode == "sbuf2":
                hAs = up([128, (COL_SPLIT // 128) * D], BF, hAs_t, "hAs")
                hBs = up([128, (COL_HI // 128) * D], BF, hBs_t, "hBs")
                sb_tabs = [hAs, hBs]

            # ---- A_all precompute: A_w = h_w @ eW1t per window ----
            A_all = cp.tile([P, NW * H], BF, tag="A_all")
            for w in range(NW):
                pA = pp.tile([P, 512], FP32, space="PSUM", tag="ps1")
                nc.tensor.matmul(pA[:, :H], lhsT=hT[:, w * WIN:(w + 1) * WIN],
                                 rhs=eW1t[:], start=True, stop=True)
                nc.vector.tensor_copy(A_all[:, w * H:(w + 1) * H], pA[:, :H])

            # ---- gather batching over regions ----
            reg_base = [0, reg_len[0]] if nhf == 2 else [0]
            nbatches = [-(-reg_len[r] // GBC) for r in range(nhf)]
            gtiles = [dict() for _ in range(nhf)]

            def emit_batch(r, b):
                if b in gtiles[r] or b >= nbatches[r]:
                    return
                b0 = b * GBC
                nb = min(GBC, reg_len[r] - b0)
                g = gp.tile([128, GBC * P], BF, tag=f"g{r}")
                kw = {}
                if mode == "sbuf2":
                    in_ap = sb_tabs[r][:]
                    kw = dict(sbuf_tokens_per_rank=128,
                              sbuf_free_dim_per_rank=256,
                              sbuf_free_dim_pad_per_rank=0,
                              sbuf_byte_offset=0)
                else:
                    in_ap = src_ap[r]
                t8 = (reg_base[r] + b0) * 8
                nc.gpsimd.dma_gather(
                    out_ap=g[:, :nb * P].rearrange("p (a n) -> p a n", a=1),
                    in_ap=in_ap,
                    idxs_ap=cidx[:, t8:t8 + nb * 8],
                    num_idxs=nb * P,
                    num_idxs_reg=nb * P,
                    elem_size=D,
                    transpose=True,
                    single_packet=False,
                    **kw,
                )
                gtiles[r][b] = g

            # window bookkeeping
            first_chunk_of_win = {}
            last_chunk_of_win = {}
            for t, (w, hf) in enumerate(chunks):
                if w not in first_chunk_of_win:
                    first_chunk_of_win[w] = t
                last_chunk_of_win[w] = t

            how_tiles = {}

            def node_phase(w, pagg):
                pagg_sb = wp.tile([WIN, H], BF, tag="paggsb")
                nc.vector.tensor_copy(pagg_sb[:], pagg[:])
                psT = pp.tile([P, 512], FP32, space="PSUM", tag="ps3")
                nc.tensor.matmul(psT[:, :WIN], lhsT=pagg_sb[:], rhs=identb[:],
                                 start=True, stop=True)
                aggT = wp.tile([H, WIN], BF, tag="aggT")
                nc.vector.tensor_copy(aggT[:], psT[:, :WIN])
                psn1 = pp.tile([P, 512], FP32, space="PSUM", tag="ps1")
                nc.tensor.matmul(psn1[:, :WIN], lhsT=nW1t[:],
                                 rhs=hT[:, w * WIN:(w + 1) * WIN],
                                 start=True, stop=False)
                nc.tensor.matmul(psn1[:, :WIN], lhsT=nW1b[:], rhs=aggT[:],
                                 start=False, stop=True)
                y1 = wp.tile([H, WIN], BF, tag="y1")
                nc.scalar.activation(y1[:], psn1[:, :WIN], act_silu,
                                     bias=nb1[:])
                psn2 = pp.tile([P, 512], FP32, space="PSUM", tag="ps2")
                nc.tensor.matmul(psn2[:, :D], lhsT=y1[:], rhs=nW2[:],
                                 start=True, stop=False)
                nc.tensor.matmul(psn2[:, :D], lhsT=ones_r[:], rhs=nb2_r[:],
                                 start=False, stop=True)
                o_sb = wp.tile([WIN, D], FP32, tag="osb")
                nc.vector.tensor_add(out=o_sb[:], in0=psn2[:, :D],
                                     in1=how_tiles.pop(w)[:])
                nc.sync.dma_start(out=out_t[w * WIN:(w + 1) * WIN, :],
                                  in_=o_sb[:])

            # ---- main loop over supertiles ----
            pagg_cur = None
            for st in range(NST):
                tl = list(range(st * 4, min(st * 4 + 4, NT)))
                nch = len(tl)
                wd = nch * P
                # gather availability + 1-batch lookahead
                for t in tl:
                    w, hf = chunks[t]
                    emit_batch(hf, reg_idx[t] // GBC)
                for t in tl:
                    w, hf = chunks[t]
                    emit_batch(hf, reg_idx[t] // GBC + 1)
                # h_own prefetch at window starts
                for t in tl:
                    w, hf = chunks[t]
                    if t == first_chunk_of_win[w]:
                        how = hop.tile([WIN, D], FP32, tag="how")
                        nc.sync.dma_start(
                            out=how[:], in_=hown_t[:, w * D:(w + 1) * D])
                        how_tiles[w] = how

                # relbc: broadcast rel along partitions via 1-row matmul
                if st % 8 == 0:
                    r2tile = rp.tile([1, 8 * 512], BF, tag="r2")
                    o0 = st * 512
                    ln = min(8 * 512, r2cols - o0)
                    nc.sync.dma_start(out=r2tile[:, :ln],
                                      in_=rel2_t[0:1, o0:o0 + ln])
                ro = (st % 8) * 512
                relbc = pp.tile([P, 512], FP32, space="PSUM", tag="relbc")
                nc.tensor.matmul(relbc[:, :wd], lhsT=ones_c[:],
                                 rhs=r2tile[0:1, ro:ro + wd],
                                 start=True, stop=True)
                # S_T[n, e] = (n == rel[e])
                S_T = wp.tile([P, 512], BF, tag="S_T")
                nc.vector.tensor_tensor(
                    out=S_T[:, :wd], in0=relbc[:, :wd],
                    in1=jconstT[:].to_broadcast([P, wd]),
                    op=mybir.AluOpType.is_equal)
                # S_eq[e, n] = (rel[e] == n), 4 chunks side by side
                S_eq = wp.tile([P, 512], BF, tag="S_eq")
                if SAFE2D:
                    for i in range(nch):
                        nc.vector.tensor_tensor(
                            out=S_eq[:, i * P:(i + 1) * P],
                            in0=rel_sb[:, tl[0] + i:tl[0] + i + 1]
                                .to_broadcast([P, WIN]),
                            in1=jconst4[:, :WIN],
                            op=mybir.AluOpType.is_equal)
                else:
                    nc.vector.tensor_tensor(
                        out=S_eq[:, :wd].rearrange("p (a n) -> p a n", a=nch),
                        in0=rel_sb[:, tl[0]:tl[0] + nch]
                            .rearrange("p (a b) -> p a b", b=1)
                            .to_broadcast([P, nch, WIN]),
                        in1=jconst4[:, :wd].rearrange("p (a n) -> p a n", a=nch),
                        op=mybir.AluOpType.is_equal)

                # ps1 = eW1b^T @ ct  +  A_w expansion
                ps1 = pp.tile([P, 512], FP32, space="PSUM", tag="ps1")
                # segments: runs of chunks contiguous in one region batch
                segs = []
                for i, t in enumerate(tl):
                    w, hf = chunks[t]
                    ri = reg_idx[t]
                    if segs and segs[-1][0] == hf and segs[-1][2] == ri \
                            and (ri % GBC) != 0:
                        segs[-1] = (hf, segs[-1][1], ri + 1, segs[-1][3] + 1)
                    else:
                        segs.append((hf, i, ri + 1, 1))
                first = True
                for (hf, i0, ri_end, ln) in segs:
                    ri0 = ri_end - ln
                    g = gtiles[hf][ri0 // GBC]
                    co = (ri0 % GBC) * P
                    nc.tensor.matmul(
                        ps1[:, i0 * P:(i0 + ln) * P], lhsT=eW1b[:],
                        rhs=g[:, co:co + ln * P], start=first, stop=False)
                    first = False
                for i, t in enumerate(tl):
                    w, hf = chunks[t]
                    nc.tensor.matmul(
                        ps1[:, i * P:(i + 1) * P],
                        lhsT=A_all[:, w * H:(w + 1) * H],
                        rhs=S_T[:, i * P:(i + 1) * P],
                        start=False, stop=(i == nch - 1))
                m1 = wp.tile([H, 512], BF, tag="m1")
                nc.scalar.activation(m1[:, :wd], ps1[:, :wd], act_silu,
                                     bias=eb1[:])
                ps2 = pp.tile([P, 512], FP32, space="PSUM", tag="ps2")
                nc.tensor.matmul(ps2[:, :wd], lhsT=eW2[:], rhs=m1[:, :wd],
                                 start=True, stop=True)
                m2 = wp.tile([H, 512], BF, tag="m2")
                nc.scalar.activation(m2[:, :wd], ps2[:, :wd], act_silu,
                                     bias=eb2[:])

                # m2 transpose (per chunk) + att preact columns
                ps3 = pp.tile([P, 512], FP32, space="PSUM", tag="ps3")
                att4 = pp.tile([P, 4], FP32, space="PSUM", tag="att4")
                for i in range(nch):
                    sl = slice(i * P, (i + 1) * P)
                    nc.tensor.matmul(ps3[:, sl], lhsT=m2[:, sl], rhs=identb[:],
                                     start=True, stop=True)
                    nc.tensor.matmul(att4[:, i:i + 1], lhsT=m2[:, sl],
                                     rhs=aW[:], start=True, stop=True)
                # sigmoid(x) = 0.5*tanh(0.5x + 0.5ab) + 0.5
                att_t = wp.tile([P, 4], FP32, tag="att_t")
                nc.scalar.activation(att_t[:, :nch], att4[:, :nch], act_tanh,
                                     bias=ab_c[:], scale=0.5)
                att_s = wp.tile([P, 4], BF, tag="att_s")
                nc.scalar.activation(att_s[:, :nch], att_t[:, :nch], act_copy,
                                     bias=0.5, scale=0.5)
                # S_att = S_eq * att (block broadcast)
                S_att = wp.tile([P, 512], BF, tag="S_att")
                if SAFE2D:
                    for i in range(nch):
                        nc.vector.tensor_tensor(
                            out=S_att[:, i * P:(i + 1) * P],
                            in0=S_eq[:, i * P:(i + 1) * P],
                            in1=att_s[:, i:i + 1].to_broadcast([P, WIN]),
                            op=mybir.AluOpType.mult)
                else:
                    nc.vector.tensor_tensor(
                        out=S_att[:, :wd].rearrange("p (a n) -> p a n", a=nch),
                        in0=S_eq[:, :wd].rearrange("p (a n) -> p a n", a=nch),
                        in1=att_s[:, :nch].rearrange("p (a b) -> p a b", b=1)
                            .to_broadcast([P, nch, WIN]),
                        op=mybir.AluOpType.mult)
                # m2T to SBUF for aggregation rhs
                m2T = wp.tile([P, 512], BF, tag="m2T")
                nc.vector.tensor_copy(m2T[:, :wd], ps3[:, :wd])

                # aggregation per chunk into the window accumulator
                for i, t in enumerate(tl):
                    w, hf = chunks[t]
                    sl = slice(i * P, (i + 1) * P)
                    if t == first_chunk_of_win[w]:
                        pagg_cur = pgp.tile([WIN, H], FP32, space="PSUM",
                                            tag="pagg")
                    nc.tensor.matmul(pagg_cur[:], lhsT=S_att[:, sl],
                                     rhs=m2T[:, sl],
                                     start=(t == first_chunk_of_win[w]),
                                     stop=(t == last_chunk_of_win[w]))
                    if t == last_chunk_of_win[w]:
                        node_phase(w, pagg_cur)
    return nc


def _make_in_maps(prep, inputs):
    eW1 = np.asarray(inputs["eW1"], np.float32)
    aW = np.asarray(inputs["aW"], np.float32)
    nW1 = np.asarray(inputs["nW1"], np.float32)
    jconst4 = np.tile(np.arange(WIN, dtype=np.float32), 4)[None, :]
    common = {
        "eW1t": eW1[:D].astype(BF16),
        "eW1b": eW1[D:].astype(BF16),
        "eW2": np.asarray(inputs["eW2"], np.float32).astype(BF16),
        "aW_col": aW.reshape(H, 1).astype(BF16),
        "nW1t": nW1[:D].astype(BF16),
        "nW1b_n": (nW1[D:] / NORM).astype(BF16),
        "nW2": np.asarray(inputs["nW2"], np.float32).astype(BF16),
        "ones_row": np.ones((1, WIN), BF16),
        "ones_col": np.ones((1, P), BF16),
        "nb2_row": np.asarray(inputs["nb2"], np.float32).reshape(1, D)
                    .astype(BF16),
        "ident_bf": np.eye(P, dtype=np.float32).astype(BF16),
        "jconst4": np.broadcast_to(jconst4, (P, 4 * WIN)).astype(BF16).copy(),
        "jconstT": np.arange(P, dtype=np.float32).reshape(P, 1),
        "eb1": np.asarray(inputs["eb1"], np.float32).reshape(H, 1),
        "eb2": np.asarray(inputs["eb2"], np.float32).reshape(H, 1),
        "nb1": np.asarray(inputs["nb1"], np.float32).reshape(H, 1),
        "ab_c": np.full((P, 1), 0.5 * float(np.asarray(inputs["ab"]).ravel()[0]),
                        dtype=np.float32),
    }
    common.update({k: v for k, v in prep["tabs"].items()})
    in_maps = []
    for k in range(NCORES):
        m = dict(common)
        m["cidx"] = np.ascontiguousarray(prep["cidx"][k])
        m["rel_sb"] = np.ascontiguousarray(prep["rel_sb"][k])
        m["rel2"] = np.ascontiguousarray(prep["rel2"][k])
        m["hT"] = np.ascontiguousarray(prep["hT"][k])
        m["h_own"] = np.ascontiguousarray(prep["h_own"][k])
        in_maps.append(m)
    return in_maps


_RUN_KW = {}


def kernel(**inputs) -> np.ndarray:
    h = np.asarray(inputs["h"], np.float32)
    prep = _preprocess(h, np.asarray(inputs["edge_index"]), MODE)

    nc = bacc.Bacc("TRN2", target_bir_lowering=False, debug=False,
                   num_devices=NCORES)
    _build(nc, prep, MODE)
    nc.compile()

    in_maps = _make_in_maps(prep, inputs)
    res = bass_utils.run_bass_kernel_spmd(
        nc, in_maps, core_ids=list(range(NCORES)), **_RUN_KW)
    out = np.empty((NPAD, D), dtype=np.float32)
    for k in range(NCORES):
        out[k * SHARD:(k + 1) * SHARD] = np.asarray(res.results[k]["out"])
    kernel._last_results = res
    return out[:N]


kernel._last_results = None


# revision 3
# speedup vs baseline: 1.9688x; 1.0199x over previous
"""GCLConv (GNN message passing) Trainium2 kernel v2 — 8-core edge-parallel.

v2 strategy (vs v1):
  - Row-side h[row] gather ELIMINATED: rows within a 128-node window are
    expanded via PE matmul (lhsT = A_w = h_w @ eW1t, rhs = S^T one-hot),
    halving SWDGE gather descriptors.
  - 512-edge supertiles (4 chunks of 128): big matmuls/activations at
    free-dim 512 amortize per-instruction overhead.
  - att folded into the aggregation S matrix (S_att = S_eq * att), so the
    edge features need no separate ef multiply; aggregation consumes the
    PE-transposed m2 directly.
  - Node phase fused per window off the PSUM aggregation (no agg buffer).
  - Gather modes: 'hbm2' dual col tables (cols split <32768 for int16),
    'hbm1' single table with sign-extended negative indices (base shifted
    by 32768 rows), 'sbuf2' SBUF-resident dual tables.
"""
import sys

sys.path.insert(0, "/opt/trn_rl_repo")

import numpy as np
import ml_dtypes

import concourse.bass as bass
import concourse.bacc as bacc
import concourse.mybir as mybir
import concourse.tile as tile
from concourse import bass_utils

BF16 = ml_dtypes.bfloat16

N = 50000
E = 800000
D = 128
H = 128
P = 128
NCORES = 8
WIN = 128
NW = 49
SHARD = WIN * NW           # 6272
NPAD = SHARD * NCORES      # 50176
COL_SPLIT = 25088
COL_HI = NPAD - COL_SPLIT  # 25088
NORM = 100.0
GBC = 32                   # chunks per gather batch (4096 indices)

MODE = "hbm2"              # 'hbm2' | 'hbm1' | 'sbuf2'
SAFE2D = False             # per-chunk 2-dim DVE builds instead of 3-dim bcast

FP32 = mybir.dt.float32
BF = mybir.dt.bfloat16
I16 = mybir.dt.int16


def _idx_layout(idx_flat: np.ndarray) -> np.ndarray:
    n = idx_flat.shape[0]
    assert n % 16 == 0
    arr = idx_flat.reshape(n // 16, 16).T.astype(np.int16)
    return np.tile(arr, (8, 1))


def _preprocess(h: np.ndarray, edge_index: np.ndarray, mode: str):
    """Group edges per core by (window[, col-half]); build tables."""
    row = np.asarray(edge_index[0], dtype=np.int64)
    col = np.asarray(edge_index[1], dtype=np.int64)
    two_regions = mode in ("hbm2", "sbuf2")

    core_of = row // SHARD
    win_of = (row % SHARD) // WIN
    half_of = (col >= COL_SPLIT).astype(np.int64) if two_regions else \
        np.zeros_like(col)
    nhf = 2 if two_regions else 1

    counts = np.zeros((NCORES, NW, nhf), dtype=np.int64)
    np.add.at(counts, (core_of, win_of, half_of), 1)
    nt = -(-counts // P)                       # ceil
    nt = nt.max(axis=0)                        # [NW, nhf] uniform over cores
    # every window needs >= 1 chunk for its aggregation group
    for w in range(NW):
        if nt[w].sum() == 0:
            nt[w, 0] = 1
    # pad total chunks to a multiple of 4 (supertiles) in the last group
    NT = int(nt.sum())
    extra = (-NT) % 4
    nt[NW - 1, nhf - 1] += extra
    NT += extra

    # processing order: (w, hf, j); region order: (hf, w, j)
    chunks = []                                # (w, hf)
    for w in range(NW):
        for hf in range(nhf):
            chunks += [(w, hf)] * int(nt[w, hf])
    assert len(chunks) == NT
    # region-local index of each chunk, and region chunk lists
    reg_len = [0] * nhf
    reg_idx = []
    for (w, hf) in chunks:
        reg_idx.append(reg_len[hf])
        reg_len[hf] += 1

    # per-core tables
    cidx = np.zeros((NCORES, 128, NT * 8), dtype=np.int16)
    rel_sb = np.zeros((NCORES, 128, NT), dtype=BF16)
    NST = NT // 4
    r2cols = NST * 512
    rel2 = np.zeros((NCORES, 1, r2cols), dtype=BF16)

    for k in range(NCORES):
        m = core_of == k
        rk = (row[m] - k * SHARD)
        ck = col[m]
        wk, hk = win_of[m], half_of[m]
        order = np.lexsort((hk, wk))
        rk, ck, wk, hk = rk[order], ck[order], wk[order], hk[order]
        cnt = np.zeros((NW, nhf), dtype=np.int64)
        np.add.at(cnt, (wk, hk), 1)
        # build padded per-chunk col/rel arrays in processing order,
        # and region-ordered col arrays for the gather table
        rel_all = np.full((NT, P), 255.0, dtype=np.float32)
        col_all = np.zeros((NT, P), dtype=np.int64)
        pos = 0
        ci = 0
        for w in range(NW):
            for hf in range(nhf):
                c = int(cnt[w, hf])
                ntg = int(nt[w, hf])
                r_g = rk[pos:pos + c] % WIN
                c_g = ck[pos:pos + c]
                pos += c
                flat_r = np.full(ntg * P, 255.0, np.float32)
                # pads must stay in-range AFTER the per-half offset
                flat_c = np.full(ntg * P, hf * COL_SPLIT, np.int64)
                flat_r[:c] = r_g
                flat_c[:c] = c_g
                rel_all[ci:ci + ntg] = flat_r.reshape(ntg, P)
                col_all[ci:ci + ntg] = flat_c.reshape(ntg, P)
                ci += ntg
        assert pos == rk.shape[0] and ci == NT
        # rel tables (processing order)
        rel_sb[k] = rel_all.T.astype(BF16)             # [128, NT]
        rel2[k, 0] = rel_all.reshape(-1).astype(BF16)
        # gather index table in REGION order
        gcols = np.zeros((NT, P), dtype=np.int64)
        # region base offsets in the table: region0 first
        base = [0, reg_len[0]] if nhf == 2 else [0]
        for t, (w, hf) in enumerate(chunks):
            gi = base[hf] + reg_idx[t]
            cc = col_all[t]
            if mode == "hbm1":
                gcols[gi] = cc - 32768
            else:
                gcols[gi] = cc - hf * COL_SPLIT
        cidx[k] = _idx_layout(gcols.reshape(-1).astype(np.int16))

    # gather source tables
    h_pad = np.zeros((NPAD, D), dtype=np.float32)
    h_pad[:N] = h
    h_bf = h_pad.astype(BF16)
    tabs = {}
    if mode == "hbm1":
        tabs["hfull"] = h_bf
    else:
        tabs["hA"] = np.ascontiguousarray(h_bf[:COL_SPLIT])
        tabs["hB"] = np.ascontiguousarray(h_bf[COL_SPLIT:])
    if mode == "sbuf2":
        def sb_layout(t):
            nr = t.shape[0] // 128
            return np.ascontiguousarray(
                t.reshape(nr, 128, D).transpose(1, 0, 2).reshape(128, nr * D))
        tabs["hA_s"] = sb_layout(tabs.pop("hA"))
        tabs["hB_s"] = sb_layout(tabs.pop("hB"))

    # node-phase per-core tables
    hsh = h_pad.reshape(NCORES, NW, WIN, D)
    h_own = np.ascontiguousarray(
        hsh.transpose(0, 2, 1, 3).reshape(NCORES, WIN, NW * D))
    hT = np.ascontiguousarray(
        hsh.transpose(0, 3, 1, 2).reshape(NCORES, D, NW * WIN)).astype(BF16)

    return dict(NT=NT, nt=nt, chunks=chunks, reg_idx=reg_idx,
                reg_len=reg_len, nhf=nhf, cidx=cidx, rel_sb=rel_sb,
                rel2=rel2, tabs=tabs, h_own=h_own, hT=hT, NST=NST,
                r2cols=r2cols)


def _build(nc: bass.Bass, prep, mode: str):
    NT, nt, chunks = prep["NT"], prep["nt"], prep["chunks"]
    reg_idx, reg_len, nhf = prep["reg_idx"], prep["reg_len"], prep["nhf"]
    NST, r2cols = prep["NST"], prep["r2cols"]
    act_silu = mybir.ActivationFunctionType.Silu
    act_tanh = mybir.ActivationFunctionType.Tanh
    act_copy = mybir.ActivationFunctionType.Copy

    dt = nc.dram_tensor
    if mode == "hbm1":
        hfull_t = dt("hfull", [NPAD, D], BF, kind="ExternalInput")
        src_ap = [hfull_t[32768:, :]]
    elif mode == "hbm2":
        hA_t = dt("hA", [COL_SPLIT, D], BF, kind="ExternalInput")
        hB_t = dt("hB", [COL_HI, D], BF, kind="ExternalInput")
        src_ap = [hA_t[:], hB_t[:]]
    else:  # sbuf2
        hAs_t = dt("hA_s", [128, (COL_SPLIT // 128) * D], BF,
                   kind="ExternalInput")
        hBs_t = dt("hB_s", [128, (COL_HI // 128) * D], BF,
                   kind="ExternalInput")
    cidx_t = dt("cidx", [128, NT * 8], I16, kind="ExternalInput")
    rel_t = dt("rel_sb", [128, NT], BF, kind="ExternalInput")
    rel2_t = dt("rel2", [1, r2cols], BF, kind="ExternalInput")
    hT_t = dt("hT", [D, NW * WIN], BF, kind="ExternalInput")
    hown_t = dt("h_own", [WIN, NW * D], FP32, kind="ExternalInput")
    # weights / consts
    eW1t_t = dt("eW1t", [D, H], BF, kind="ExternalInput")
    eW1b_t = dt("eW1b", [D, H], BF, kind="ExternalInput")
    eW2_t = dt("eW2", [H, H], BF, kind="ExternalInput")
    aW_t = dt("aW_col", [H, 1], BF, kind="ExternalInput")
    nW1t_t = dt("nW1t", [D, H], BF, kind="ExternalInput")
    nW1b_t = dt("nW1b_n", [H, H], BF, kind="ExternalInput")
    nW2_t = dt("nW2", [H, D], BF, kind="ExternalInput")
    ones_r_t = dt("ones_row", [1, WIN], BF, kind="ExternalInput")
    ones_c_t = dt("ones_col", [1, P], BF, kind="ExternalInput")
    nb2_t = dt("nb2_row", [1, D], BF, kind="ExternalInput")
    identb_t = dt("ident_bf", [P, P], BF, kind="ExternalInput")
    jconst4_t = dt("jconst4", [P, 4 * WIN], BF, kind="ExternalInput")
    jconstT_t = dt("jconstT", [P, 1], FP32, kind="ExternalInput")
    eb1_t = dt("eb1", [H, 1], FP32, kind="ExternalInput")
    eb2_t = dt("eb2", [H, 1], FP32, kind="ExternalInput")
    nb1_t = dt("nb1", [H, 1], FP32, kind="ExternalInput")
    ab_t = dt("ab_c", [P, 1], FP32, kind="ExternalInput")
    out_t = dt("out", [SHARD, D], FP32, kind="ExternalOutput")

    with tile.TileContext(nc) as tc:
        with (
            tc.tile_pool(name="const", bufs=1) as cp,
            tc.tile_pool(name="gat", bufs=3 if nhf == 1 else 2) as gp,
            tc.tile_pool(name="work", bufs=3) as wp,
            tc.tile_pool(name="how", bufs=2) as hop,
            tc.tile_pool(name="r2s", bufs=2) as rp,
            tc.tile_pool(name="pp", bufs=1, space="PSUM") as pp,
            tc.tile_pool(name="ppd", bufs=2, space="PSUM") as ppd,
            tc.tile_pool(name="pagg", bufs=2, space="PSUM") as pgp,
        ):
            def up(shape, dtype, src, tag):
                t = cp.tile(shape, dtype, tag=tag)
                nc.sync.dma_start(out=t[:], in_=src[:])
                return t

            cidx = up([128, NT * 8], I16, cidx_t, "cidx")
            rel_sb = up([128, NT], BF, rel_t, "rel")
            hT = up([D, NW * WIN], BF, hT_t, "hT")
            eW1t = up([D, H], BF, eW1t_t, "eW1t")
            eW1b = up([D, H], BF, eW1b_t, "eW1b")
            eW2 = up([H, H], BF, eW2_t, "eW2")
            aW = up([H, 1], BF, aW_t, "aW")
            nW1t = up([D, H], BF, nW1t_t, "nW1t")
            nW1b = up([H, H], BF, nW1b_t, "nW1b")
            nW2 = up([H, D], BF, nW2_t, "nW2")
            ones_r = up([1, WIN], BF, ones_r_t, "onesr")
            ones_c = up([1, P], BF, ones_c_t, "onesc")
            nb2_r = up([1, D], BF, nb2_t, "nb2")
            identb = up([P, P], BF, identb_t, "identb")
            jconst4 = up([P, 4 * WIN], BF, jconst4_t, "jc4")
            jconstT = up([P, 1], FP32, jconstT_t, "jcT")
            eb1 = up([H, 1], FP32, eb1_t, "eb1")
            eb2 = up([H, 1], FP32, eb2_t, "eb2")
            nb1 = up([H, 1], FP32, nb1_t, "nb1")
            ab_c = up([P, 1], FP32, ab_t, "ab")
            if mode == "sbuf2":
                hAs = up([128, (COL_SPLIT // 128) * D], BF, hAs_t, "hAs")
                hBs = up([128, (COL_HI // 128) * D], BF, hBs_t, "hBs")
                sb_tabs = [hAs, hBs]

            # ---- A_all precompute: A_w = h_w @ eW1t per window ----
            A_all = cp.tile([P, NW * H], BF, tag="A_all")
            for w in range(NW):
                pA = ppd.tile([P, 512], FP32, space="PSUM", tag="ps1")
                nc.tensor.matmul(pA[:, :H], lhsT=hT[:, w * WIN:(w + 1) * WIN],
                                 rhs=eW1t[:], start=True, stop=True)
                nc.vector.tensor_copy(A_all[:, w * H:(w + 1) * H], pA[:, :H])

            # ---- gather batching over regions ----
            reg_base = [0, reg_len[0]] if nhf == 2 else [0]
            nbatches = [-(-reg_len[r] // GBC) for r in range(nhf)]
            gtiles = [dict() for _ in range(nhf)]

            def emit_batch(r, b):
                if b in gtiles[r] or b >= nbatches[r]:
                    return
                b0 = b * GBC
                nb = min(GBC, reg_len[r] - b0)
                g = gp.tile([128, GBC * P], BF, tag=f"g{r}")
                kw = {}
                if mode == "sbuf2":
                    in_ap = sb_tabs[r][:]
                    kw = dict(sbuf_tokens_per_rank=128,
                              sbuf_free_dim_per_rank=256,
                              sbuf_free_dim_pad_per_rank=0,
                              sbuf_byte_offset=0)
                else:
                    in_ap = src_ap[r]
                t8 = (reg_base[r] + b0) * 8
                nc.gpsimd.dma_gather(
                    out_ap=g[:, :nb * P].rearrange("p (a n) -> p a n", a=1),
                    in_ap=in_ap,
                    idxs_ap=cidx[:, t8:t8 + nb * 8],
                    num_idxs=nb * P,
                    num_idxs_reg=nb * P,
                    elem_size=D,
                    transpose=True,
                    single_packet=False,
                    **kw,
                )
                gtiles[r][b] = g

            # window bookkeeping
            first_chunk_of_win = {}
            last_chunk_of_win = {}
            for t, (w, hf) in enumerate(chunks):
                if w not in first_chunk_of_win:
                    first_chunk_of_win[w] = t
                last_chunk_of_win[w] = t

            how_tiles = {}

            def node_phase(w, pagg):
                pagg_sb = wp.tile([WIN, H], BF, tag="paggsb")
                nc.vector.tensor_copy(pagg_sb[:], pagg[:])
                psT = pp.tile([P, 512], FP32, space="PSUM", tag="ps3")
                nc.tensor.matmul(psT[:, :WIN], lhsT=pagg_sb[:], rhs=identb[:],
                                 start=True, stop=True)
                aggT = wp.tile([H, WIN], BF, tag="aggT")
                nc.vector.tensor_copy(aggT[:], psT[:, :WIN])
                psn1 = ppd.tile([P, 512], FP32, space="PSUM", tag="ps1")
                nc.tensor.matmul(psn1[:, :WIN], lhsT=nW1t[:],
                                 rhs=hT[:, w * WIN:(w + 1) * WIN],
                                 start=True, stop=False)
                nc.tensor.matmul(psn1[:, :WIN], lhsT=nW1b[:], rhs=aggT[:],
                                 start=False, stop=True)
                y1 = wp.tile([H, WIN], BF, tag="y1")
                nc.scalar.activation(y1[:], psn1[:, :WIN], act_silu,
                                     bias=nb1[:])
                psn2 = pp.tile([P, 512], FP32, space="PSUM", tag="ps2")
                nc.tensor.matmul(psn2[:, :D], lhsT=y1[:], rhs=nW2[:],
                                 start=True, stop=False)
                nc.tensor.matmul(psn2[:, :D], lhsT=ones_r[:], rhs=nb2_r[:],
                                 start=False, stop=True)
                o_sb = wp.tile([WIN, D], FP32, tag="osb")
                nc.vector.tensor_add(out=o_sb[:], in0=psn2[:, :D],
                                     in1=how_tiles.pop(w)[:])
                nc.sync.dma_start(out=out_t[w * WIN:(w + 1) * WIN, :],
                                  in_=o_sb[:])

            # ---- main loop over supertiles ----
            pagg_cur = None
            for st in range(NST):
                tl = list(range(st * 4, min(st * 4 + 4, NT)))
                nch = len(tl)
                wd = nch * P
                # gather availability + 1-batch lookahead
                for t in tl:
                    w, hf = chunks[t]
                    emit_batch(hf, reg_idx[t] // GBC)
                for t in tl:
                    w, hf = chunks[t]
                    emit_batch(hf, reg_idx[t] // GBC + 1)
                # h_own prefetch at window starts
                for t in tl:
                    w, hf = chunks[t]
                    if t == first_chunk_of_win[w]:
                        how = hop.tile([WIN, D], FP32, tag="how")
                        nc.sync.dma_start(
                            out=how[:], in_=hown_t[:, w * D:(w + 1) * D])
                        how_tiles[w] = how

                # relbc: broadcast rel along partitions via 1-row matmul
                if st % 8 == 0:
                    r2tile = rp.tile([1, 8 * 512], BF, tag="r2")
                    o0 = st * 512
                    ln = min(8 * 512, r2cols - o0)
                    nc.sync.dma_start(out=r2tile[:, :ln],
                                      in_=rel2_t[0:1, o0:o0 + ln])
                ro = (st % 8) * 512
                relbc = pp.tile([P, 512], FP32, space="PSUM", tag="relbc")
                nc.tensor.matmul(relbc[:, :wd], lhsT=ones_c[:],
                                 rhs=r2tile[0:1, ro:ro + wd],
                                 start=True, stop=True)
                # S_T[n, e] = (n == rel[e])
                S_T = wp.tile([P, 512], BF, tag="S_T")
                nc.vector.tensor_tensor(
                    out=S_T[:, :wd], in0=relbc[:, :wd],
                    in1=jconstT[:].to_broadcast([P, wd]),
                    op=mybir.AluOpType.is_equal)
                # S_eq[e, n] = (rel[e] == n), 4 chunks side by side
                S_eq = wp.tile([P, 512], BF, tag="S_eq")
                if SAFE2D:
                    for i in range(nch):
                        nc.vector.tensor_tensor(
                            out=S_eq[:, i * P:(i + 1) * P],
                            in0=rel_sb[:, tl[0] + i:tl[0] + i + 1]
                                .to_broadcast([P, WIN]),
                            in1=jconst4[:, :WIN],
                            op=mybir.AluOpType.is_equal)
                else:
                    nc.vector.tensor_tensor(
                        out=S_eq[:, :wd].rearrange("p (a n) -> p a n", a=nch),
                        in0=rel_sb[:, tl[0]:tl[0] + nch]
                            .rearrange("p (a b) -> p a b", b=1)
                            .to_broadcast([P, nch, WIN]),
                        in1=jconst4[:, :wd].rearrange("p (a n) -> p a n", a=nch),
                        op=mybir.AluOpType.is_equal)

                # ps1 = eW1b^T @ ct  +  A_w expansion
                ps1 = ppd.tile([P, 512], FP32, space="PSUM", tag="ps1")
                # segments: runs of chunks contiguous in one region batch
                segs = []
                for i, t in enumerate(tl):
                    w, hf = chunks[t]
                    ri = reg_idx[t]
                    if segs and segs[-1][0] == hf and segs[-1][2] == ri \
                            and (ri % GBC) != 0:
                        segs[-1] = (hf, segs[-1][1], ri + 1, segs[-1][3] + 1)
                    else:
                        segs.append((hf, i, ri + 1, 1))
                first = True
                for (hf, i0, ri_end, ln) in segs:
                    ri0 = ri_end - ln
                    g = gtiles[hf][ri0 // GBC]
                    co = (ri0 % GBC) * P
                    nc.tensor.matmul(
                        ps1[:, i0 * P:(i0 + ln) * P], lhsT=eW1b[:],
                        rhs=g[:, co:co + ln * P], start=first, stop=False)
                    first = False
                for i, t in enumerate(tl):
                    w, hf = chunks[t]
                    nc.tensor.matmul(
                        ps1[:, i * P:(i + 1) * P],
                        lhsT=A_all[:, w * H:(w + 1) * H],
                        rhs=S_T[:, i * P:(i + 1) * P],
                        start=False, stop=(i == nch - 1))
                m1 = wp.tile([H, 512], BF, tag="m1")
                nc.scalar.activation(m1[:, :wd], ps1[:, :wd], act_silu,
                                     bias=eb1[:])
                ps2 = pp.tile([P, 512], FP32, space="PSUM", tag="ps2")
                nc.tensor.matmul(ps2[:, :wd], lhsT=eW2[:], rhs=m1[:, :wd],
                                 start=True, stop=True)
                m2 = wp.tile([H, 512], BF, tag="m2")
                nc.scalar.activation(m2[:, :wd], ps2[:, :wd], act_silu,
                                     bias=eb2[:])

                # m2 transpose (per chunk) + att preact columns
                ps3 = pp.tile([P, 512], FP32, space="PSUM", tag="ps3")
                att4 = pp.tile([P, 4], FP32, space="PSUM", tag="att4")
                for i in range(nch):
                    sl = slice(i * P, (i + 1) * P)
                    nc.tensor.matmul(ps3[:, sl], lhsT=m2[:, sl], rhs=identb[:],
                                     start=True, stop=True)
                    nc.tensor.matmul(att4[:, i:i + 1], lhsT=m2[:, sl],
                                     rhs=aW[:], start=True, stop=True)
                # sigmoid(x) = 0.5*tanh(0.5x + 0.5ab) + 0.5
                att_t = wp.tile([P, 4], FP32, tag="att_t")
                nc.scalar.activation(att_t[:, :nch], att4[:, :nch], act_tanh,
                                     bias=ab_c[:], scale=0.5)
                att_s = wp.tile([P, 4], BF, tag="att_s")
                nc.scalar.activation(att_s[:, :nch], att_t[:, :nch], act_copy,
                                     bias=0.5, scale=0.5)
                # S_att = S_eq * att (block broadcast)
                S_att = wp.tile([P, 512], BF, tag="S_att")
                if SAFE2D:
                    for i in range(nch):
                        nc.vector.tensor_tensor(
                            out=S_att[:, i * P:(i + 1) * P],
                            in0=S_eq[:, i * P:(i + 1) * P],
                            in1=att_s[:, i:i + 1].to_broadcast([P, WIN]),
                            op=mybir.AluOpType.mult)
                else:
                    nc.vector.tensor_tensor(
                        out=S_att[:, :wd].rearrange("p (a n) -> p a n", a=nch),
                        in0=S_eq[:, :wd].rearrange("p (a n) -> p a n", a=nch),
                        in1=att_s[:, :nch].rearrange("p (a b) -> p a b", b=1)
                            .to_broadcast([P, nch, WIN]),
                        op=mybir.AluOpType.mult)
                # m2T to SBUF for aggregation rhs
                m2T = wp.tile([P, 512], BF, tag="m2T")
                nc.vector.tensor_copy(m2T[:, :wd], ps3[:, :wd])

                # aggregation per chunk into the window accumulator
                for i, t in enumerate(tl):
                    w, hf = chunks[t]
                    sl = slice(i * P, (i + 1) * P)
                    if t == first_chunk_of_win[w]:
                        pagg_cur = pgp.tile([WIN, H], FP32, space="PSUM",
                                            tag="pagg")
                    nc.tensor.matmul(pagg_cur[:], lhsT=S_att[:, sl],
                                     rhs=m2T[:, sl],
                                     start=(t == first_chunk_of_win[w]),
                                     stop=(t == last_chunk_of_win[w]))
                    if t == last_chunk_of_win[w]:
                        node_phase(w, pagg_cur)
    return nc


def _make_in_maps(prep, inputs):
    eW1 = np.asarray(inputs["eW1"], np.float32)
    aW = np.asarray(inputs["aW"], np.float32)
    nW1 = np.asarray(inputs["nW1"], np.float32)
    jconst4 = np.tile(np.arange(WIN, dtype=np.float32), 4)[None, :]
    common = {
        "eW1t": eW1[:D].astype(BF16),
        "eW1b": eW1[D:].astype(BF16),
        "eW2": np.asarray(inputs["eW2"], np.float32).astype(BF16),
        "aW_col": aW.reshape(H, 1).astype(BF16),
        "nW1t": nW1[:D].astype(BF16),
        "nW1b_n": (nW1[D:] / NORM).astype(BF16),
        "nW2": np.asarray(inputs["nW2"], np.float32).astype(BF16),
        "ones_row": np.ones((1, WIN), BF16),
        "ones_col": np.ones((1, P), BF16),
        "nb2_row": np.asarray(inputs["nb2"], np.float32).reshape(1, D)
                    .astype(BF16),
        "ident_bf": np.eye(P, dtype=np.float32).astype(BF16),
        "jconst4": np.broadcast_to(jconst4, (P, 4 * WIN)).astype(BF16).copy(),
        "jconstT": np.arange(P, dtype=np.float32).reshape(P, 1),
        "eb1": np.asarray(inputs["eb1"], np.float32).reshape(H, 1),
        "eb2": np.asarray(inputs["eb2"], np.float32).reshape(H, 1),
        "nb1": np.asarray(inputs["nb1"], np.float32).reshape(H, 1),
        "ab_c": np.full((P, 1), 0.5 * float(np.asarray(inputs["ab"]).ravel()[0]),
                        dtype=np.float32),
    }
    common.update({k: v for k, v in prep["tabs"].items()})
    in_maps = []
    for k in range(NCORES):
        m = dict(common)
        m["cidx"] = np.ascontiguousarray(prep["cidx"][k])
        m["rel_sb"] = np.ascontiguousarray(prep["rel_sb"][k])
        m["rel2"] = np.ascontiguousarray(prep["rel2"][k])
        m["hT"] = np.ascontiguousarray(prep["hT"][k])
        m["h_own"] = np.ascontiguousarray(prep["h_own"][k])
        in_maps.append(m)
    return in_maps


_RUN_KW = {}


def kernel(**inputs) -> np.ndarray:
    h = np.asarray(inputs["h"], np.float32)
    prep = _preprocess(h, np.asarray(inputs["edge_index"]), MODE)

    nc = bacc.Bacc("TRN2", target_bir_lowering=False, debug=False,
                   num_devices=NCORES)
    _build(nc, prep, MODE)
    nc.compile()

    in_maps = _make_in_maps(prep, inputs)
    res = bass_utils.run_bass_kernel_spmd(
        nc, in_maps, core_ids=list(range(NCORES)), **_RUN_KW)
    out = np.empty((NPAD, D), dtype=np.float32)
    for k in range(NCORES):
        out[k * SHARD:(k + 1) * SHARD] = np.asarray(res.results[k]["out"])
    kernel._last_results = res
    return out[:N]


kernel._last_results = None
